# revision 25
# baseline (speedup 1.0000x reference)
"""Trainium2 Bass kernel for AdversarialLogLikelihoodLossLayer.

Per (b,t): negative log-likelihood of a C=40-dim Gaussian
    nll = 0.5*(d^T Sigma^-1 d + logdet Sigma + C*log(2pi)),  d = y_true - mu
summed over T, meaned over B.

Algorithm: batched bordered LDL^T elimination on the Vector engine.
  M = [[Sigma, d], [d^T, 0]]  (41x41, symmetric)
  After eliminating 40 columns (rank-1 Schur updates, no sqrt needed):
    - the 40 pivots p_j satisfy logdet Sigma = sum log p_j
    - the final corner M[40,40] = -d^T Sigma^-1 d
Layout: 128 independent problems across SBUF partitions, the 41x41 matrix
row-major in the free dim. Each rank-1 update is 2 DVE ops:
    P   = (w_bcast_i * (1/pivot)) * w_bcast_k     (scalar_tensor_tensor)
    blk = blk - P                                  (tensor_tensor)
using stride-0 broadcast APs of the pivot row.

Data parallel over 8 NeuronCores: 32000 problems -> 4000/core, padded to
4096 = 32 tiles of 128 with identity problems (which contribute exactly 0).
"""

import sys
from contextlib import ExitStack

import numpy as np

sys.path.insert(0, "/opt/trn_rl_repo")

import concourse.bacc as bacc  # noqa: E402
import concourse.bass as bass  # noqa: E402
from concourse import mybir  # noqa: E402
from concourse.bass_utils import run_bass_kernel_spmd  # noqa: E402
from concourse.tile import TileContext  # noqa: E402

B, T, C = 64, 500, 40
CP1 = C + 1
N_CORES = 8
NPROB = B * T
PER_CORE = NPROB // N_CORES  # 4000
PAD = 4096                   # per-core padded problem count
NTILES_FULL = PAD // 128     # 32

F32 = mybir.dt.float32
OP = mybir.AluOpType
LOG_2PI = float(np.log(2.0 * np.pi))
SPLIT_THRESHOLDS = (8, 15, 22, 30, 37)
ABUFS = 4
PBUFS = 2
SBUFS = 6
U_ON_GPSIMD = False


def _emit_tile(nc, pools, yp, yt, vout, t):
    """Emit the full processing of one 128-problem tile."""
    apool, ppool, spool = pools
    lo = t * 128
    A = apool.tile([128, CP1, CP1], F32, tag="A")
    D = spool.tile([128, C], F32, tag="D")
    # Sigma rows + mu land directly in M[0:40, :]: y_pred row-major
    # [40,41] matches M's first 40 rows, mu in column 40.
    nc.sync.dma_start(
        out=A[:, 0:C, :],
        in_=yp[lo : lo + 128, :].rearrange("p (i k) -> p i k", i=C),
    )
    nc.sync.dma_start(out=D[:], in_=yt[lo : lo + 128, :])
    # d = y_true - mu
    nc.vector.tensor_tensor(out=D[:], in0=D[:], in1=A[:, 0:C, C], op=OP.subtract)
    nc.vector.tensor_copy(out=A[:, 0:C, C], in_=D[:])   # border column
    nc.vector.tensor_copy(out=A[:, C, 0:C], in_=D[:])   # border row
    nc.vector.memset(A[:, C, C : C + 1], 0.0)           # corner

    for j in range(C):
        m = CP1 - 1 - j  # trailing block size
        R = spool.tile([128, 1], F32, tag="R")
        nc.vector.reciprocal(out=R[:], in_=A[:, j, j : j + 1])
        row = A[:, j, j + 1 : CP1]                      # [128, m] pivot row
        v_i = row[:, :, None].broadcast_to([128, m, m])  # w[i] over (i,k)
        v_k = row[:, None, :].broadcast_to([128, m, m])  # w[k] over (i,k)
        Pt = ppool.tile([128, m, m], F32, tag="P")
        nc.vector.scalar_tensor_tensor(
            out=Pt[:], in0=v_i, scalar=R[:], in1=v_k, op0=OP.mult, op1=OP.mult
        )
        nc.vector.tensor_tensor(
            out=A[:, j + 1 :, j + 1 :], in0=A[:, j + 1 :, j + 1 :], in1=Pt[:],
            op=OP.subtract,
        )

    # v = sum_j log(pivot_j) - corner   (corner = -d^T Sigma^-1 d)
    a = A[:]
    diag = bass.AP(tensor=a.tensor, offset=a.offset, ap=[a.ap[0], [CP1 + 1, C]])
    LOGT = spool.tile([128, C], F32, tag="LOG")
    S = spool.tile([128, 1], F32, tag="S")
    nc.scalar.activation(
        out=LOGT[:], in_=diag, func=mybir.ActivationFunctionType.Ln,
        accum_out=S[:],
    )
    V = spool.tile([128, 1], F32, tag="V")
    nc.vector.tensor_tensor(out=V[:], in0=S[:], in1=A[:, C, C : C + 1], op=OP.subtract)
    nc.sync.dma_start(out=vout[lo : lo + 128, :], in_=V[:])


def _make_pools(tc, ctx, G: int = 1):
    abufs = ABUFS if G <= 4 else 2
    sbufs = SBUFS if G <= 4 else 3
    apool = ctx.enter_context(tc.tile_pool(name="A", bufs=abufs))
    ppool = ctx.enter_context(tc.tile_pool(name="P", bufs=PBUFS))
    spool = ctx.enter_context(tc.tile_pool(name="small", bufs=sbufs))
    return apool, ppool, spool


def _emit_tile_packed(nc, pools, yp, yt, vout, t, G, big_eng=None):
    """One tile = G*128 problems: G matrices packed along the free dim of
    each partition. Outputs one partially-summed value per partition.

    The border ROW (d^T) is never materialized: at step j the update only
    writes rows j+1..39 x cols j+1..40. Row j is final after step j, so the
    end-state matrix holds every pivot row; with the saved reciprocals the
    quadratic form is q = sum_j A[j,40]^2 / p_j.
    """
    apool, ppool, spool = pools
    big = big_eng if big_eng is not None else nc.vector
    lo = t * G * 128
    A = apool.tile([128, G, C, CP1], F32, tag="A")          # rows 0..39 only
    D = spool.tile([128, G, C], F32, tag="D")
    RA = spool.tile([128, G, C], F32, tag="RA")             # 1/pivot per step
    nc.sync.dma_start(
        out=A[:],
        in_=yp[lo : lo + G * 128, :].rearrange("(g p) (i k) -> p g i k", g=G, i=C),
    )
    nc.sync.dma_start(
        out=D[:], in_=yt[lo : lo + G * 128, :].rearrange("(g p) c -> p g c", g=G)
    )
    # border column: d = y_true - mu  (mu is already in column 40)
    nc.vector.tensor_tensor(out=D[:], in0=D[:], in1=A[:, :, 0:C, C], op=OP.subtract)
    nc.vector.tensor_copy(out=A[:, :, 0:C, C], in_=D[:])

    U = spool.tile([128, G, C], F32, tag="U")
    for j in range(C):
        m = C - j  # trailing columns j+1..40 (incl. border col) = m
        nc.vector.reciprocal(out=RA[:, :, j : j + 1], in_=A[:, :, j, j : j + 1])
        if j == C - 1:
            break  # no trailing rows left; row 39 final
        row = A[:, :, j, j + 1 : CP1]                       # [128, G, m]
        ueng = nc.gpsimd if U_ON_GPSIMD else big
        ueng.tensor_tensor(                                 # u = w / pivot
            out=U[:, :, 0:m], in0=row,
            in1=RA[:, :, j : j + 1].broadcast_to([128, G, m]), op=OP.mult,
        )
        mr = m - 1                                          # rows j+1..39
        # Only entries (i, k>=i) plus the border column are ever read later;
        # cover the upper trapezoid with row-blocks whose columns start at
        # the block's first row (bounding rectangles).
        nb = 1 + sum(mr >= t for t in SPLIT_THRESHOLDS)
        bounds = [rs * mr // nb for rs in range(nb)] + [mr]
        for b in range(nb):
            rs, re = bounds[b], bounds[b + 1]
            nrows = re - rs
            v_i = U[:, :, rs:re, None].broadcast_to([128, G, nrows, m - rs])
            v_k = row[:, :, None, rs:m].broadcast_to([128, G, nrows, m - rs])
            Pt = ppool.tile([128, G, nrows, m - rs], F32, tag="P")
            big.tensor_tensor(out=Pt[:], in0=v_i, in1=v_k, op=OP.mult)
            blk = A[:, :, j + 1 + rs : j + 1 + re, j + 1 + rs :]
            big.tensor_tensor(out=blk, in0=blk, in1=Pt[:], op=OP.subtract)

    # per-partition partial sum over g: sum_j log(p_j) + sum_j dcol_j^2/p_j
    a = A[:]
    diag = bass.AP(
        tensor=a.tensor, offset=a.offset,
        ap=[a.ap[0], [C * CP1, G], [CP1 + 1, C]],
    )
    LOGT = spool.tile([128, G, C], F32, tag="LOG")
    S = spool.tile([128, 1], F32, tag="S")
    nc.scalar.activation(
        out=LOGT[:], in_=diag, func=mybir.ActivationFunctionType.Ln,
        accum_out=S[:],
    )
    dcol = A[:, :, 0:C, C]                                  # final border col
    SQ = spool.tile([128, G, C], F32, tag="SQ")
    nc.vector.tensor_tensor(out=SQ[:], in0=dcol, in1=dcol, op=OP.mult)
    Q = spool.tile([128, 1], F32, tag="Q")
    nc.vector.scalar_tensor_tensor(
        out=SQ[:], in0=SQ[:], scalar=1.0, in1=RA[:], op0=OP.mult, op1=OP.mult,
        accum_out=Q[:],
    )
    V = spool.tile([128, 1], F32, tag="V")
    nc.vector.tensor_tensor(out=V[:], in0=S[:], in1=Q[:], op=OP.add)
    nc.sync.dma_start(out=vout[t * 128 : (t + 1) * 128, :], in_=V[:])


def build(ntiles: int = NTILES_FULL) -> bass.Bass:
    nprob = ntiles * 128
    nc = bacc.Bacc("TRN2", target_bir_lowering=False)
    yp = nc.dram_tensor("y_pred", [nprob, C * CP1], F32, kind="ExternalInput")
    yt = nc.dram_tensor("y_true", [nprob, C], F32, kind="ExternalInput")
    vout = nc.dram_tensor("v_out", [nprob, 1], F32, kind="ExternalOutput")

    with TileContext(nc) as tc, ExitStack() as ctx:
        pools = _make_pools(tc, ctx)
        for t in range(ntiles):
            _emit_tile(nc, pools, yp, yt, vout, t)
    if not nc.is_finalized():
        nc.finalize()
    return nc


def build_loop(body_tiles: int, reps: int) -> bass.Bass:
    """Timing amplifier: process the same `body_tiles` tiles `reps` times
    inside a For_i loop (static addressing; WAW across reps is fine)."""
    nprob = body_tiles * 128
    nc = bacc.Bacc("TRN2", target_bir_lowering=False)
    yp = nc.dram_tensor("y_pred", [nprob, C * CP1], F32, kind="ExternalInput")
    yt = nc.dram_tensor("y_true", [nprob, C], F32, kind="ExternalInput")
    vout = nc.dram_tensor("v_out", [nprob, 1], F32, kind="ExternalOutput")

    with TileContext(nc) as tc, ExitStack() as ctx:
        pools = _make_pools(tc, ctx)

        def body(i, unroll=1):
            for t in range(body_tiles):
                _emit_tile(nc, pools, yp, yt, vout, t)

        with tc.For_i(0, reps, 1) as i:
            body(i)
    if not nc.is_finalized():
        nc.finalize()
    return nc


def build2(ntiles: int, G: int) -> bass.Bass:
    """Packed variant: each tile covers G*128 problems."""
    nprob = ntiles * G * 128
    nc = bacc.Bacc("TRN2", target_bir_lowering=False)
    yp = nc.dram_tensor("y_pred", [nprob, C * CP1], F32, kind="ExternalInput")
    yt = nc.dram_tensor("y_true", [nprob, C], F32, kind="ExternalInput")
    vout = nc.dram_tensor("v_out", [ntiles * 128, 1], F32, kind="ExternalOutput")

    with TileContext(nc) as tc, ExitStack() as ctx:
        pools = _make_pools(tc, ctx, G)
        for t in range(ntiles):
            _emit_tile_packed(nc, pools, yp, yt, vout, t, G)
    if not nc.is_finalized():
        nc.finalize()
    return nc


def build_loop2(body_tiles: int, reps: int, G: int, gp_every: int = 0) -> bass.Bass:
    """gp_every=k: every k-th tile runs its big ops on GPSIMD (0 = never)."""
    nprob = body_tiles * G * 128
    nc = bacc.Bacc("TRN2", target_bir_lowering=False)
    yp = nc.dram_tensor("y_pred", [nprob, C * CP1], F32, kind="ExternalInput")
    yt = nc.dram_tensor("y_true", [nprob, C], F32, kind="ExternalInput")
    vout = nc.dram_tensor("v_out", [body_tiles * 128, 1], F32, kind="ExternalOutput")

    with TileContext(nc) as tc, ExitStack() as ctx:
        pools = _make_pools(tc, ctx, G)

        def body(i, unroll=1):
            for t in range(body_tiles):
                eng = nc.gpsimd if (gp_every and t % gp_every == gp_every - 1) else None
                _emit_tile_packed(nc, pools, yp, yt, vout, t, G, big_eng=eng)

        with tc.For_i(0, reps, 1) as i:
            body(i)
    if not nc.is_finalized():
        nc.finalize()
    return nc


_CACHE: dict = {}


def _pad_rows(n_pad: int) -> tuple[np.ndarray, np.ndarray]:
    """Identity problems: Sigma=I, mu=0, y_true=0 -> v contribution exactly 0."""
    row = np.concatenate([np.eye(C, dtype=np.float32), np.zeros((C, 1), np.float32)], axis=1)
    return (
        np.tile(row.reshape(1, -1), (n_pad, 1)),
        np.zeros((n_pad, C), np.float32),
    )


G_PACK = 4
NTILES_PACKED = PAD // (G_PACK * 128)  # 8


def kernel(y_true: np.ndarray, y_pred: np.ndarray) -> np.ndarray:
    ypf = np.ascontiguousarray(y_pred.reshape(NPROB, C * CP1).astype(np.float32, copy=False))
    ytf = np.ascontiguousarray(y_true.reshape(NPROB, C).astype(np.float32, copy=False))

    if "nc" not in _CACHE:
        _CACHE["nc"] = build2(NTILES_PACKED, G_PACK)
    nc = _CACHE["nc"]

    pad_p, pad_t = _pad_rows(PAD - PER_CORE)
    in_maps = []
    for c in range(N_CORES):
        sl = slice(c * PER_CORE, (c + 1) * PER_CORE)
        in_maps.append({
            "y_pred": np.concatenate([ypf[sl], pad_p], axis=0),
            "y_true": np.concatenate([ytf[sl], pad_t], axis=0),
        })

    res = run_bass_kernel_spmd(nc, in_maps, core_ids=list(range(N_CORES)))
    # v_out rows are per-partition partial sums (padding contributes 0)
    v = np.concatenate([r["v_out"][:, 0] for r in res.results])
    loss = 0.5 * float(np.sum(v, dtype=np.float64)) / B + T * 0.5 * C * LOG_2PI
    return np.float32(loss)


# revision 26
# speedup vs baseline: 485.0473x; 485.0473x over previous
"""Trainium2 Bass kernel for AdversarialLogLikelihoodLossLayer.

Per (b,t): negative log-likelihood of a C=40-dim Gaussian
    nll = 0.5*(d^T Sigma^-1 d + logdet Sigma + C*log(2pi)),  d = y_true - mu
summed over T, meaned over B -> scalar.

Algorithm: batched bordered LDL^T (no sqrt, no pivoting; Sigma is SPD and
well-conditioned). Per problem form M = [Sigma | d] (40x41; the d border
column replaces mu in-place after one subtract; the border row is never
materialized). 40 rank-1 Schur eliminations give pivots p_j with
logdet = sum_j log p_j, and since row j is final after step j, the end-state
border column holds w_j[40], so q = d^T Sigma^-1 d = sum_j M[j,40]^2 / p_j
using the saved pivot reciprocals.

Layout: 128 problems across SBUF partitions x G=4 matrices packed along the
free dim of each partition (amortizes per-op overhead; the per-partition
scalar limit of tensor_scalar ops forces a separate u = w/p scale op).
Each step's update runs as 1-6 DVE tensor_tensor pairs (product via
stride-0-broadcast APs of the pivot row, then subtract) covering only the
upper trapezoid + border column — the strictly-lower triangle is never read
by later steps, so it is skipped via row-blocks whose columns start at the
block's first row. Pivot logs are summed in one ScalarE Ln+accum op.

Data parallel over 8 NeuronCores: 32000 problems -> 4000/core, padded to
4096 = 8 packed tiles with identity problems (which contribute exactly 0);
per-partition partial sums are reduced on the host in float64.
"""

import sys
from contextlib import ExitStack

import numpy as np

sys.path.insert(0, "/opt/trn_rl_repo")

import concourse.bacc as bacc  # noqa: E402
import concourse.bass as bass  # noqa: E402
from concourse import mybir  # noqa: E402
from concourse.bass_utils import run_bass_kernel_spmd  # noqa: E402
from concourse.tile import TileContext  # noqa: E402

B, T, C = 64, 500, 40
CP1 = C + 1
N_CORES = 8
NPROB = B * T
PER_CORE = NPROB // N_CORES  # 4000
PAD = 4096                   # per-core padded problem count
NTILES_FULL = PAD // 128     # 32

F32 = mybir.dt.float32
OP = mybir.AluOpType
LOG_2PI = float(np.log(2.0 * np.pi))
SPLIT_THRESHOLDS = (8, 15, 22, 30, 37)
ABUFS = 4
PBUFS = 2
SBUFS = 6
U_ON_GPSIMD = False


def _emit_tile(nc, pools, yp, yt, vout, t):
    """Emit the full processing of one 128-problem tile."""
    apool, ppool, spool = pools
    lo = t * 128
    A = apool.tile([128, CP1, CP1], F32, tag="A")
    D = spool.tile([128, C], F32, tag="D")
    # Sigma rows + mu land directly in M[0:40, :]: y_pred row-major
    # [40,41] matches M's first 40 rows, mu in column 40.
    nc.sync.dma_start(
        out=A[:, 0:C, :],
        in_=yp[lo : lo + 128, :].rearrange("p (i k) -> p i k", i=C),
    )
    nc.sync.dma_start(out=D[:], in_=yt[lo : lo + 128, :])
    # d = y_true - mu
    nc.vector.tensor_tensor(out=D[:], in0=D[:], in1=A[:, 0:C, C], op=OP.subtract)
    nc.vector.tensor_copy(out=A[:, 0:C, C], in_=D[:])   # border column
    nc.vector.tensor_copy(out=A[:, C, 0:C], in_=D[:])   # border row
    nc.vector.memset(A[:, C, C : C + 1], 0.0)           # corner

    for j in range(C):
        m = CP1 - 1 - j  # trailing block size
        R = spool.tile([128, 1], F32, tag="R")
        nc.vector.reciprocal(out=R[:], in_=A[:, j, j : j + 1])
        row = A[:, j, j + 1 : CP1]                      # [128, m] pivot row
        v_i = row[:, :, None].broadcast_to([128, m, m])  # w[i] over (i,k)
        v_k = row[:, None, :].broadcast_to([128, m, m])  # w[k] over (i,k)
        Pt = ppool.tile([128, m, m], F32, tag="P")
        nc.vector.scalar_tensor_tensor(
            out=Pt[:], in0=v_i, scalar=R[:], in1=v_k, op0=OP.mult, op1=OP.mult
        )
        nc.vector.tensor_tensor(
            out=A[:, j + 1 :, j + 1 :], in0=A[:, j + 1 :, j + 1 :], in1=Pt[:],
            op=OP.subtract,
        )

    # v = sum_j log(pivot_j) - corner   (corner = -d^T Sigma^-1 d)
    a = A[:]
    diag = bass.AP(tensor=a.tensor, offset=a.offset, ap=[a.ap[0], [CP1 + 1, C]])
    LOGT = spool.tile([128, C], F32, tag="LOG")
    S = spool.tile([128, 1], F32, tag="S")
    nc.scalar.activation(
        out=LOGT[:], in_=diag, func=mybir.ActivationFunctionType.Ln,
        accum_out=S[:],
    )
    V = spool.tile([128, 1], F32, tag="V")
    nc.vector.tensor_tensor(out=V[:], in0=S[:], in1=A[:, C, C : C + 1], op=OP.subtract)
    nc.sync.dma_start(out=vout[lo : lo + 128, :], in_=V[:])


def _make_pools(tc, ctx, G: int = 1):
    abufs = ABUFS if G <= 4 else 2
    sbufs = SBUFS if G <= 4 else 3
    apool = ctx.enter_context(tc.tile_pool(name="A", bufs=abufs))
    ppool = ctx.enter_context(tc.tile_pool(name="P", bufs=PBUFS))
    spool = ctx.enter_context(tc.tile_pool(name="small", bufs=sbufs))
    return apool, ppool, spool


def _emit_tile_packed(nc, pools, yp, yt, vout, t, G, big_eng=None):
    """One tile = G*128 problems: G matrices packed along the free dim of
    each partition. Outputs one partially-summed value per partition.

    The border ROW (d^T) is never materialized: at step j the update only
    writes rows j+1..39 x cols j+1..40. Row j is final after step j, so the
    end-state matrix holds every pivot row; with the saved reciprocals the
    quadratic form is q = sum_j A[j,40]^2 / p_j.
    """
    apool, ppool, spool = pools
    big = big_eng if big_eng is not None else nc.vector
    lo = t * G * 128
    A = apool.tile([128, G, C, CP1], F32, tag="A")          # rows 0..39 only
    D = spool.tile([128, G, C], F32, tag="D")
    RA = spool.tile([128, G, C], F32, tag="RA")             # 1/pivot per step
    nc.sync.dma_start(
        out=A[:],
        in_=yp[lo : lo + G * 128, :].rearrange("(g p) (i k) -> p g i k", g=G, i=C),
    )
    nc.sync.dma_start(
        out=D[:], in_=yt[lo : lo + G * 128, :].rearrange("(g p) c -> p g c", g=G)
    )
    # border column: d = y_true - mu  (mu is already in column 40)
    nc.vector.tensor_tensor(out=D[:], in0=D[:], in1=A[:, :, 0:C, C], op=OP.subtract)
    nc.vector.tensor_copy(out=A[:, :, 0:C, C], in_=D[:])

    U = spool.tile([128, G, C], F32, tag="U")
    for j in range(C):
        m = C - j  # trailing columns j+1..40 (incl. border col) = m
        nc.vector.reciprocal(out=RA[:, :, j : j + 1], in_=A[:, :, j, j : j + 1])
        if j == C - 1:
            break  # no trailing rows left; row 39 final
        row = A[:, :, j, j + 1 : CP1]                       # [128, G, m]
        ueng = nc.gpsimd if U_ON_GPSIMD else big
        ueng.tensor_tensor(                                 # u = w / pivot
            out=U[:, :, 0:m], in0=row,
            in1=RA[:, :, j : j + 1].broadcast_to([128, G, m]), op=OP.mult,
        )
        mr = m - 1                                          # rows j+1..39
        # Only entries (i, k>=i) plus the border column are ever read later;
        # cover the upper trapezoid with row-blocks whose columns start at
        # the block's first row (bounding rectangles).
        nb = 1 + sum(mr >= t for t in SPLIT_THRESHOLDS)
        bounds = [rs * mr // nb for rs in range(nb)] + [mr]
        for b in range(nb):
            rs, re = bounds[b], bounds[b + 1]
            nrows = re - rs
            v_i = U[:, :, rs:re, None].broadcast_to([128, G, nrows, m - rs])
            v_k = row[:, :, None, rs:m].broadcast_to([128, G, nrows, m - rs])
            Pt = ppool.tile([128, G, nrows, m - rs], F32, tag="P")
            big.tensor_tensor(out=Pt[:], in0=v_i, in1=v_k, op=OP.mult)
            blk = A[:, :, j + 1 + rs : j + 1 + re, j + 1 + rs :]
            big.tensor_tensor(out=blk, in0=blk, in1=Pt[:], op=OP.subtract)

    # per-partition partial sum over g: sum_j log(p_j) + sum_j dcol_j^2/p_j
    a = A[:]
    diag = bass.AP(
        tensor=a.tensor, offset=a.offset,
        ap=[a.ap[0], [C * CP1, G], [CP1 + 1, C]],
    )
    LOGT = spool.tile([128, G, C], F32, tag="LOG")
    S = spool.tile([128, 1], F32, tag="S")
    nc.scalar.activation(
        out=LOGT[:], in_=diag, func=mybir.ActivationFunctionType.Ln,
        accum_out=S[:],
    )
    dcol = A[:, :, 0:C, C]                                  # final border col
    SQ = spool.tile([128, G, C], F32, tag="SQ")
    nc.vector.tensor_tensor(out=SQ[:], in0=dcol, in1=dcol, op=OP.mult)
    Q = spool.tile([128, 1], F32, tag="Q")
    nc.vector.scalar_tensor_tensor(
        out=SQ[:], in0=SQ[:], scalar=1.0, in1=RA[:], op0=OP.mult, op1=OP.mult,
        accum_out=Q[:],
    )
    V = spool.tile([128, 1], F32, tag="V")
    nc.vector.tensor_tensor(out=V[:], in0=S[:], in1=Q[:], op=OP.add)
    nc.sync.dma_start(out=vout[t * 128 : (t + 1) * 128, :], in_=V[:])


def build(ntiles: int = NTILES_FULL) -> bass.Bass:
    nprob = ntiles * 128
    nc = bacc.Bacc("TRN2", target_bir_lowering=False)
    yp = nc.dram_tensor("y_pred", [nprob, C * CP1], F32, kind="ExternalInput")
    yt = nc.dram_tensor("y_true", [nprob, C], F32, kind="ExternalInput")
    vout = nc.dram_tensor("v_out", [nprob, 1], F32, kind="ExternalOutput")

    with TileContext(nc) as tc, ExitStack() as ctx:
        pools = _make_pools(tc, ctx)
        for t in range(ntiles):
            _emit_tile(nc, pools, yp, yt, vout, t)
    if not nc.is_finalized():
        nc.finalize()
    return nc


def build_loop(body_tiles: int, reps: int) -> bass.Bass:
    """Timing amplifier: process the same `body_tiles` tiles `reps` times
    inside a For_i loop (static addressing; WAW across reps is fine)."""
    nprob = body_tiles * 128
    nc = bacc.Bacc("TRN2", target_bir_lowering=False)
    yp = nc.dram_tensor("y_pred", [nprob, C * CP1], F32, kind="ExternalInput")
    yt = nc.dram_tensor("y_true", [nprob, C], F32, kind="ExternalInput")
    vout = nc.dram_tensor("v_out", [nprob, 1], F32, kind="ExternalOutput")

    with TileContext(nc) as tc, ExitStack() as ctx:
        pools = _make_pools(tc, ctx)

        def body(i, unroll=1):
            for t in range(body_tiles):
                _emit_tile(nc, pools, yp, yt, vout, t)

        with tc.For_i(0, reps, 1) as i:
            body(i)
    if not nc.is_finalized():
        nc.finalize()
    return nc


def build2(ntiles: int, G: int) -> bass.Bass:
    """Packed variant: each tile covers G*128 problems."""
    nprob = ntiles * G * 128
    nc = bacc.Bacc("TRN2", target_bir_lowering=False)
    yp = nc.dram_tensor("y_pred", [nprob, C * CP1], F32, kind="ExternalInput")
    yt = nc.dram_tensor("y_true", [nprob, C], F32, kind="ExternalInput")
    vout = nc.dram_tensor("v_out", [ntiles * 128, 1], F32, kind="ExternalOutput")

    with TileContext(nc) as tc, ExitStack() as ctx:
        pools = _make_pools(tc, ctx, G)
        for t in range(ntiles):
            _emit_tile_packed(nc, pools, yp, yt, vout, t, G)
    if not nc.is_finalized():
        nc.finalize()
    return nc


def build_loop2(body_tiles: int, reps: int, G: int, gp_every: int = 0) -> bass.Bass:
    """gp_every=k: every k-th tile runs its big ops on GPSIMD (0 = never)."""
    nprob = body_tiles * G * 128
    nc = bacc.Bacc("TRN2", target_bir_lowering=False)
    yp = nc.dram_tensor("y_pred", [nprob, C * CP1], F32, kind="ExternalInput")
    yt = nc.dram_tensor("y_true", [nprob, C], F32, kind="ExternalInput")
    vout = nc.dram_tensor("v_out", [body_tiles * 128, 1], F32, kind="ExternalOutput")

    with TileContext(nc) as tc, ExitStack() as ctx:
        pools = _make_pools(tc, ctx, G)

        def body(i, unroll=1):
            for t in range(body_tiles):
                eng = nc.gpsimd if (gp_every and t % gp_every == gp_every - 1) else None
                _emit_tile_packed(nc, pools, yp, yt, vout, t, G, big_eng=eng)

        with tc.For_i(0, reps, 1) as i:
            body(i)
    if not nc.is_finalized():
        nc.finalize()
    return nc


_CACHE: dict = {}


def _pad_rows(n_pad: int) -> tuple[np.ndarray, np.ndarray]:
    """Identity problems: Sigma=I, mu=0, y_true=0 -> v contribution exactly 0."""
    row = np.concatenate([np.eye(C, dtype=np.float32), np.zeros((C, 1), np.float32)], axis=1)
    return (
        np.tile(row.reshape(1, -1), (n_pad, 1)),
        np.zeros((n_pad, C), np.float32),
    )


G_PACK = 4
NTILES_PACKED = PAD // (G_PACK * 128)  # 8


def kernel(y_true: np.ndarray, y_pred: np.ndarray) -> np.ndarray:
    ypf = np.ascontiguousarray(y_pred.reshape(NPROB, C * CP1).astype(np.float32, copy=False))
    ytf = np.ascontiguousarray(y_true.reshape(NPROB, C).astype(np.float32, copy=False))

    if "nc" not in _CACHE:
        _CACHE["nc"] = build2(NTILES_PACKED, G_PACK)
    nc = _CACHE["nc"]

    pad_p, pad_t = _pad_rows(PAD - PER_CORE)
    in_maps = []
    for c in range(N_CORES):
        sl = slice(c * PER_CORE, (c + 1) * PER_CORE)
        in_maps.append({
            "y_pred": np.concatenate([ypf[sl], pad_p], axis=0),
            "y_true": np.concatenate([ytf[sl], pad_t], axis=0),
        })

    res = run_bass_kernel_spmd(nc, in_maps, core_ids=list(range(N_CORES)))
    # v_out rows are per-partition partial sums (padding contributes 0)
    v = np.concatenate([r["v_out"][:, 0] for r in res.results])
    loss = 0.5 * float(np.sum(v, dtype=np.float64)) / B + T * 0.5 * C * LOG_2PI
    return np.float32(loss)
